# revision 2
# baseline (speedup 1.0000x reference)
"""GATNE model (attention over edge types + ragged segment-mean + FC) on 8
Trainium2 NeuronCores via Bass/Tile — count-matrix matmul version.

Strategy (self-contained; hardcoded for the spec shapes): data-parallel over
the batch for phase 1, segment-sharded for phase 2, per the sharding hint.
Both gathers/segment-reductions are reformulated as dense count-matrix
matmuls (scattered 256B SWDGE gathers measure ~13 GB/s/core on this part,
while dense streams exceed 100 GB/s):
  Phase 1 (per 128-sample type-homogeneous tile): the neighbor sum
  nte[b,t,:] = sum_c cnt[t,c][node,b]^T @ node_type_table[chunk c] runs on
  the PE with host-built fp8e4m3 count matrices (counts are small ints,
  exact in fp8; PE takes fp8 lhsT x bf16 rhs). Only the per-sample
  node_embeddings lookup stays a SWDGE gather (512B bf16 rows). Attention
  via bf16 PE matmuls + ACT softmax, row L2-normalize, bf16 output table.
  Host relays the 8 shards into one bf16 table (untimed, like the baseline).
  Phase 2 (256 segments per core): seg_sum[s,:] = sum_c cnt2[c][row,s]^T @
  lne[chunk c] on the PE, with the lne table + fp8 count matrices streamed
  densely; then mean + L2-normalize, bf16 FC, f32 l2norm output.
"""
import contextlib
import sys

sys.path.insert(0, "/opt/trn_rl_repo")

import ml_dtypes
import numpy as np

import concourse.bacc as bacc
import concourse.bass as bass
import concourse.mybir as mybir
import concourse.tile as tile
from concourse.bass_utils import run_bass_kernel_spmd
from concourse.masks import make_identity

F32 = mybir.dt.float32
BF16 = mybir.dt.bfloat16
I16 = mybir.dt.int16
AF = mybir.ActivationFunctionType
ALU = mybir.AluOpType
NPBF16 = ml_dtypes.bfloat16
# Count matrices ride as fp8e4m3: the counts are small integers (<= 10),
# exactly representable, and the PE accepts fp8 lhsT with bf16 rhs.
CNT_DT = mybir.dt.float8e4
CNT_NP = ml_dtypes.float8_e4m3

N_CORES = 8
NUM_NODES = 2500
T = 4
EMB = 256
U = 64
DIM_A = 32
EMBED_SIZE = 512
NEIGH = 10
B = NUM_NODES * T
NUM_SMS = 64
MAX_REGION = 32
NSEG = NUM_SMS * MAX_REGION  # 2048
SEG_PER_CORE = NSEG // N_CORES  # 256
E_TOTAL = 131072
P = 128
NCH1 = 20  # node chunks (2560 >= 2500)


def _wrap16(flat):
    n = flat.shape[0]
    assert n % 16 == 0
    a = flat.reshape(n // 16, 16).T.astype(np.int16)
    return np.tile(a, (8, 1))


def _split_drain_waits(nc, max_waits=1):
    for bb in nc.main_func.blocks:
        out = []
        for ins in bb.instructions:
            si = ins.sync_info
            if (
                type(ins).__name__ == "InstDrain"
                and si is not None
                and si.on_wait is not None
                and len(si.on_wait) > max_waits
            ):
                waits = list(si.on_wait)
                extra, keep = waits[:-max_waits], waits[-max_waits:]
                for i in range(0, len(extra), max_waits):
                    d = mybir.InstDrain(
                        name=nc.get_next_instruction_name(),
                        ins=[],
                        outs=[],
                        bass_is_fusable=False,
                    )
                    d.engine = ins.engine
                    d.sync_info = mybir.SyncInfo(
                        on_wait=extra[i : i + max_waits], on_update=[]
                    )
                    out.append(d)
                ins.sync_info = mybir.SyncInfo(
                    on_wait=keep, on_update=list(si.on_update or [])
                )
            out.append(ins)
        bb.instructions = out


# ---------------------------------------------------------------------------
# Phase-1 program builder
# ---------------------------------------------------------------------------
def build_phase1(TPC, reps=1):
    nc = bacc.Bacc("TRN2", debug=False)
    ttabs = nc.dram_tensor("ttabs", [P, NCH1 * T * U], BF16, kind="ExternalInput")
    cnts = nc.dram_tensor(
        "cnts", [P, TPC * T * NCH1 * P], CNT_DT, kind="ExternalInput"
    )
    ntab = nc.dram_tensor("ntab", [NUM_NODES, EMB], BF16, kind="ExternalInput")
    neidx = nc.dram_tensor("neidx", [P, TPC * 8], I16, kind="ExternalInput")
    s1w = nc.dram_tensor("s1w", [U, TPC * DIM_A], BF16, kind="ExternalInput")
    s2w = nc.dram_tensor("s2w", [DIM_A, TPC], BF16, kind="ExternalInput")
    ww = nc.dram_tensor("ww", [U, TPC * EMB], BF16, kind="ExternalInput")
    lne_out = nc.dram_tensor("lne", [P, TPC * EMB], BF16, kind="ExternalOutput")

    with tile.TileContext(nc) as tc:
        with (
            tc.tile_pool(name="const", bufs=1) as cpool,
            tc.tile_pool(name="cnt", bufs=3) as cntpool,
            tc.tile_pool(name="work", bufs=2) as wpool,
            tc.tile_pool(name="ps_nte", bufs=2, space="PSUM") as ps_nte,
            tc.tile_pool(name="ps_tp", bufs=1, space="PSUM") as ps_tp,
            tc.tile_pool(name="ps_h", bufs=1, space="PSUM") as ps_h,
            tc.tile_pool(name="ps_lg", bufs=1, space="PSUM") as ps_lg,
            tc.tile_pool(name="ps_dl", bufs=2, space="PSUM") as ps_dl,
        ):
            identb = cpool.tile([P, P], BF16)
            make_identity(nc, identb[:])
            ident = cpool.tile([P, P], F32)
            make_identity(nc, ident[:])
            ttabs_sb = cpool.tile([P, NCH1, T, U], BF16)
            nc.sync.dma_start(
                out=ttabs_sb[:].rearrange("p c t u -> p (c t u)"), in_=ttabs[:]
            )
            s1_sb = cpool.tile([U, TPC * DIM_A], BF16)
            nc.sync.dma_start(out=s1_sb[:], in_=s1w[:])
            s2_sb = cpool.tile([DIM_A, TPC], BF16)
            nc.sync.dma_start(out=s2_sb[:], in_=s2w[:])
            w_sb = cpool.tile([U, TPC * EMB], BF16)
            nc.sync.dma_start(out=w_sb[:], in_=ww[:])
            ne_idx_sb = cpool.tile([P, TPC * 8], I16)
            nc.sync.dma_start(out=ne_idx_sb[:], in_=neidx[:])
            lne_sb = cpool.tile([P, TPC * EMB], BF16)

            with tc.For_i(0, reps, 1) if reps > 1 else contextlib.nullcontext():
                # per-sample node_embeddings gather (bf16, 512B rows)
                ne_g = cpool.tile([P, TPC, EMB], BF16, tag="ne_g")
                nc.gpsimd.dma_gather(
                    ne_g[:], ntab[:], ne_idx_sb[:], TPC * P, TPC * P, EMB,
                    single_packet=False,
                )
                for j in range(TPC):
                    # ---- counts for this tile: [p_node, (t,c), b]
                    cnt = cntpool.tile([P, T * NCH1, P], CNT_DT, tag="cnt")
                    nc.sync.dma_start(
                        out=cnt[:].rearrange("p a b -> p (a b)"),
                        in_=cnts[:, j * (T * NCH1 * P) : (j + 1) * (T * NCH1 * P)],
                    )
                    # ---- nte[b, (t,u)] via count matmuls
                    nte_ps = ps_nte.tile([P, T * U], F32, tag="nte_ps")
                    for t in range(T):
                        for c in range(NCH1):
                            nc.tensor.matmul(
                                out=nte_ps[:, t * U : (t + 1) * U],
                                lhsT=cnt[:, t * NCH1 + c, :],
                                rhs=ttabs_sb[:, c, t, :],
                                start=(c == 0),
                                stop=(c == NCH1 - 1),
                            )
                    # bf16 copy for transposes/matmuls (ACT)
                    nteb = wpool.tile([P, T * U], BF16, tag="nteb")
                    nc.scalar.activation(nteb[:], nte_ps[:], AF.Copy)
                    # ---- transpose nte -> per-type [64u, 128b] blocks (bf16)
                    pt = ps_tp.tile([U, T * P], BF16, tag="pt")
                    for t in range(T):
                        nc.tensor.transpose(
                            pt[:, t * P : (t + 1) * P],
                            nteb[:, t * U : (t + 1) * U],
                            identb[:],
                        )
                    ntet = wpool.tile([U, T * P], BF16, tag="ntet")
                    nc.vector.tensor_copy(out=ntet[:], in_=pt[:])

                    # ---- h = tanh(nte @ S1): one bf16 matmul [32a, (t,b)]
                    h_ps = ps_h.tile([DIM_A, T * P], F32, tag="h_ps")
                    nc.tensor.matmul(
                        out=h_ps[:],
                        lhsT=s1_sb[:, j * DIM_A : (j + 1) * DIM_A],
                        rhs=ntet[:],
                        start=True,
                        stop=True,
                    )
                    h = wpool.tile([DIM_A, T * P], BF16, tag="h")
                    nc.scalar.activation(h[:], h_ps[:], AF.Tanh)

                    # ---- logits [128b, 4t]
                    lg_ps = ps_lg.tile([P, T], F32, tag="lg_ps")
                    for t in range(T):
                        nc.tensor.matmul(
                            out=lg_ps[:, t : t + 1],
                            lhsT=h[:, t * P : (t + 1) * P],
                            rhs=s2_sb[:, j : j + 1],
                            start=True,
                            stop=True,
                        )
                    # ---- softmax over 4 type slots (f32)
                    nmax = wpool.tile([P, 1], F32, tag="nmax")
                    nc.vector.tensor_reduce(
                        out=nmax[:],
                        in_=lg_ps[:],
                        axis=mybir.AxisListType.X,
                        op=ALU.max,
                        negate=True,
                    )
                    ex = wpool.tile([P, T], F32, tag="ex")
                    nc.scalar.activation(ex[:], lg_ps[:], AF.Exp, bias=nmax[:, 0:1])
                    ssum = wpool.tile([P, 1], F32, tag="ssum")
                    nc.vector.tensor_reduce(
                        out=ssum[:], in_=ex[:], axis=mybir.AxisListType.X, op=ALU.add
                    )
                    rs = wpool.tile([P, 1], F32, tag="rs")
                    nc.vector.reciprocal(rs[:], ssum[:])
                    att = wpool.tile([P, T], F32, tag="att")
                    nc.vector.tensor_scalar_mul(att[:], ex[:], rs[:, 0:1])

                    # ---- agg[b,u] = sum_t att[b,t] * nte[b,t,u]
                    ntev = nte_ps[:].rearrange("p (t u) -> p t u", t=T)
                    attb = att[:, :, None].to_broadcast([P, T, U])
                    tmp = wpool.tile([P, T, U], F32, tag="tmp")
                    nc.vector.tensor_tensor(
                        out=tmp[:], in0=ntev, in1=attb, op=ALU.mult
                    )
                    agg = wpool.tile([P, U], F32, tag="agg")
                    tmpv = tmp[:].rearrange("p t u -> p u t")
                    nc.vector.tensor_reduce(
                        out=agg[:], in_=tmpv, axis=mybir.AxisListType.X, op=ALU.add
                    )
                    # ---- aggT [64, 128] bf16
                    at_ps = ps_lg.tile([U, P], F32, tag="at_ps")
                    nc.tensor.transpose(at_ps[:], agg[:], ident[:])
                    aggT = wpool.tile([U, P], BF16, tag="aggT")
                    nc.vector.tensor_copy(out=aggT[:], in_=at_ps[:])

                    # ---- delta = aggT.T @ W -> [128b, 256e]
                    dl_ps = ps_dl.tile([P, EMB], F32, tag="dl_ps")
                    nc.tensor.matmul(
                        out=dl_ps[:],
                        lhsT=aggT[:],
                        rhs=w_sb[:, j * EMB : (j + 1) * EMB],
                        start=True,
                        stop=True,
                    )
                    # ---- ne_new = node_embed + delta; L2 normalize rows
                    nen = wpool.tile([P, EMB], F32, tag="nen")
                    nc.vector.tensor_add(out=nen[:], in0=dl_ps[:], in1=ne_g[:, j, :])
                    sq = wpool.tile([P, EMB], F32, tag="sq")
                    ss = wpool.tile([P, 1], F32, tag="ss")
                    nc.scalar.activation(sq[:], nen[:], AF.Square, accum_out=ss[:])
                    nrm = wpool.tile([P, 1], F32, tag="nrm")
                    nc.scalar.activation(nrm[:], ss[:], AF.Sqrt)
                    nc.vector.tensor_scalar_max(nrm[:], nrm[:], 1e-12)
                    inv = wpool.tile([P, 1], F32, tag="inv")
                    nc.vector.reciprocal(inv[:], nrm[:])
                    nc.vector.tensor_scalar_mul(
                        lne_sb[:, j * EMB : (j + 1) * EMB], nen[:], inv[:, 0:1]
                    )
            nc.sync.dma_start(out=lne_out[:], in_=lne_sb[:])

    nc.compile()
    _split_drain_waits(nc)
    return nc


# ---------------------------------------------------------------------------
# Phase-2 program builder
# ---------------------------------------------------------------------------
def build_phase2(NCH, reps=1):
    """NCH: lne row chunks of 128 (= TT). Segment-sum via dense count-matrix
    matmuls; lne + counts streamed in groups of GRP chunks."""
    GRP = 8
    nc = bacc.Bacc("TRN2", debug=False)
    lne = nc.dram_tensor("lne", [NCH * P, EMB], BF16, kind="ExternalInput")
    cnt2 = nc.dram_tensor("cnt2", [P, NCH * 2 * P], CNT_DT, kind="ExternalInput")
    recip = nc.dram_tensor("recip", [P, 2], F32, kind="ExternalInput")
    fcwt = nc.dram_tensor("fcwt", [EMB, EMBED_SIZE], BF16, kind="ExternalInput")
    fcb = nc.dram_tensor("fcb", [P, EMBED_SIZE], F32, kind="ExternalInput")
    out = nc.dram_tensor("out", [SEG_PER_CORE, EMBED_SIZE], F32, kind="ExternalOutput")

    lne_v = lne[:].rearrange("(c p) e -> p c e", p=P)  # [128, NCH, 256]

    with tile.TileContext(nc) as tc:
        with (
            tc.tile_pool(name="const", bufs=1) as cpool,
            tc.tile_pool(name="lg", bufs=3) as lgpool,
            tc.tile_pool(name="cg", bufs=3) as cgpool,
            tc.tile_pool(name="work", bufs=2) as wpool,
            tc.tile_pool(name="ps_acc", bufs=2, space="PSUM") as ps_acc,
            tc.tile_pool(name="ps_tp", bufs=2, space="PSUM") as ps_tp,
            tc.tile_pool(name="ps_fc", bufs=2, space="PSUM") as ps_fc,
        ):
            identb = cpool.tile([P, P], BF16)
            make_identity(nc, identb[:])
            recip_sb = cpool.tile([P, 2], F32)
            nc.sync.dma_start(out=recip_sb[:], in_=recip[:])
            fcwt0 = cpool.tile([P, EMBED_SIZE], BF16, tag="fcwt0")
            fcwt1 = cpool.tile([P, EMBED_SIZE], BF16, tag="fcwt1")
            fcwt_sb = [fcwt0, fcwt1]
            for i in range(2):
                nc.sync.dma_start(out=fcwt_sb[i][:], in_=fcwt[i * P : (i + 1) * P, :])
            fcb_sb = cpool.tile([P, EMBED_SIZE], F32)
            nc.sync.dma_start(out=fcb_sb[:], in_=fcb[:])
            eps8 = cpool.tile([P, 1], F32)
            nc.vector.memset(eps8[:], 1e-8)

            with tc.For_i(0, reps, 1) if reps > 1 else contextlib.nullcontext():
                acc0 = ps_acc.tile([P, EMB], F32, tag="acc0")
                acc1 = ps_acc.tile([P, EMB], F32, tag="acc1")
                acc = [acc0, acc1]
                nc.vector.memset(acc[0][:], 0.0)
                nc.vector.memset(acc[1][:], 0.0)
                done = 0
                while done < NCH:
                    ng = min(GRP, NCH - done)
                    lg = lgpool.tile([P, GRP, EMB], BF16, tag="lg")
                    nc.sync.dma_start(
                        out=lg[:, :ng, :], in_=lne_v[:, done : done + ng, :]
                    )
                    cg = cgpool.tile([P, GRP, 2, P], CNT_DT, tag="cg")
                    nc.sync.dma_start(
                        out=cg[:, :ng, :, :].rearrange("p a b s -> p (a b s)"),
                        in_=cnt2[:, done * 2 * P : (done + ng) * 2 * P],
                    )
                    for cc in range(ng):
                        for h in range(2):
                            nc.tensor.matmul(
                                out=acc[h][:],
                                lhsT=cg[:, cc, h, :],
                                rhs=lg[:, cc, :],
                                start=False,
                                stop=False,
                                skip_group_check=True,
                            )
                    done += ng
                # ---- mean + normalize -> smn [2][128, 256] bf16
                smn = []
                for half in range(2):
                    mean = wpool.tile([P, EMB], F32, tag=f"mean{half}")
                    nc.vector.tensor_scalar_mul(
                        mean[:], acc[half][:], recip_sb[:, half : half + 1]
                    )
                    sq = wpool.tile([P, EMB], F32, tag="p2sq")
                    ss = wpool.tile([P, 1], F32, tag="p2ss")
                    nc.scalar.activation(sq[:], mean[:], AF.Square, accum_out=ss[:])
                    nrm = wpool.tile([P, 1], F32, tag="p2nrm")
                    nc.scalar.activation(nrm[:], ss[:], AF.Sqrt)
                    nc.vector.tensor_scalar_max(nrm[:], nrm[:], 1e-12)
                    inv = wpool.tile([P, 1], F32, tag="p2inv")
                    nc.vector.reciprocal(inv[:], nrm[:])
                    sm = wpool.tile([P, EMB], BF16, tag=f"smn{half}")
                    nc.vector.tensor_scalar_mul(sm[:], mean[:], inv[:, 0:1])
                    smn.append(sm)
                # ---- transpose smn -> smnT [emb-half][128, 256(seg)] bf16
                smnT = []
                for eh in range(2):
                    tp = ps_tp.tile([P, 2 * P], BF16, tag="tp")
                    for half in range(2):
                        nc.tensor.transpose(
                            tp[:, half * P : (half + 1) * P],
                            smn[half][:, eh * P : (eh + 1) * P],
                            identb[:],
                        )
                    st = wpool.tile([P, 2 * P], BF16, tag=f"smnT{eh}")
                    nc.vector.tensor_copy(out=st[:], in_=tp[:])
                    smnT.append(st)
                # ---- FC + bias + l2norm
                for m in range(2):
                    fc_ps = ps_fc.tile([P, EMBED_SIZE], F32, tag="fc_ps")
                    for kh in range(2):
                        nc.tensor.matmul(
                            out=fc_ps[:],
                            lhsT=smnT[kh][:, m * P : (m + 1) * P],
                            rhs=fcwt_sb[kh][:],
                            start=(kh == 0),
                            stop=(kh == 1),
                        )
                    xx = wpool.tile([P, EMBED_SIZE], F32, tag="xx")
                    nc.vector.tensor_add(out=xx[:], in0=fc_ps[:], in1=fcb_sb[:])
                    sq = wpool.tile([P, EMBED_SIZE], F32, tag="p3sq")
                    ss = wpool.tile([P, 1], F32, tag="p3ss")
                    nc.scalar.activation(sq[:], xx[:], AF.Square, accum_out=ss[:])
                    nrm = wpool.tile([P, 1], F32, tag="p3nrm")
                    nc.scalar.activation(nrm[:], ss[:], AF.Sqrt, bias=eps8[:, 0:1])
                    nc.vector.tensor_scalar_add(nrm[:], nrm[:], 1e-8)
                    inv = wpool.tile([P, 1], F32, tag="p3inv")
                    nc.vector.reciprocal(inv[:], nrm[:])
                    res = wpool.tile([P, EMBED_SIZE], F32, tag="res")
                    nc.vector.tensor_scalar_mul(res[:], xx[:], inv[:, 0:1])
                    nc.sync.dma_start(
                        out=out[m * P : (m + 1) * P, :], in_=res[:]
                    )

    nc.compile()
    _split_drain_waits(nc)
    return nc


# ---------------------------------------------------------------------------
# Host-side orchestration
# ---------------------------------------------------------------------------
def _phase1_prep(train_inputs, train_types, node_neigh):
    order = np.argsort(train_types, kind="stable")
    ts = train_types[order]
    tiles_s, tiles_t = [], []
    for t in range(T):
        idx_t = order[ts == t]
        if len(idx_t) == 0:
            continue
        n_tiles = -(-len(idx_t) // P)
        padded = np.concatenate(
            [idx_t, np.repeat(idx_t[-1:], n_tiles * P - len(idx_t))]
        )
        for jj in range(n_tiles):
            tiles_s.append(padded[jj * P : (jj + 1) * P])
            tiles_t.append(t)
    while len(tiles_s) % N_CORES:
        tiles_s.append(tiles_s[-1])
        tiles_t.append(tiles_t[-1])
    sample_mat = np.stack(tiles_s)
    tile_type = np.asarray(tiles_t)
    TT = sample_mat.shape[0]
    TPC = TT // N_CORES

    flat = sample_mat.reshape(-1)
    slot_of_sample = np.zeros(B, np.int64)
    slot_of_sample[flat[::-1]] = np.arange(TT * P)[::-1]
    return sample_mat, tile_type, TPC, slot_of_sample


def _phase1_inmaps(inputs, sample_mat, tile_type, TPC):
    node_embeddings = np.asarray(inputs["node_embeddings"], np.float32)
    node_type_embeddings = np.asarray(inputs["node_type_embeddings"], np.float32)
    trans_weights = np.asarray(inputs["trans_weights"], np.float32)
    trans_weights_s1 = np.asarray(inputs["trans_weights_s1"], np.float32)
    trans_weights_s2 = np.asarray(inputs["trans_weights_s2"], np.float32)
    train_inputs = np.asarray(inputs["train_inputs"])
    node_neigh = np.asarray(inputs["node_neigh"])

    # ttabs[p, c, t, u] = node_type_embeddings[c*128+p, t, u] (0-pad)
    nte_pad = np.zeros((NCH1 * P, T, U), np.float32)
    nte_pad[:NUM_NODES] = node_type_embeddings
    ttabs = np.ascontiguousarray(
        nte_pad.reshape(NCH1, P, T, U).transpose(1, 0, 2, 3).reshape(P, -1)
    ).astype(NPBF16)
    ntab = node_embeddings.astype(NPBF16)

    in_maps = []
    for k in range(N_CORES):
        smp = sample_mat[k * TPC : (k + 1) * TPC]  # [TPC, 128]
        ct = tile_type[k * TPC : (k + 1) * TPC]
        ne_flat = train_inputs[smp].reshape(-1)
        ne_idx = _wrap16(ne_flat)
        nn_t = node_neigh[smp]  # [TPC, 128, 4, 10]
        # counts A[j, t, c, p_node, b]
        A = np.zeros((TPC, T, NCH1, P, P), np.uint8)
        j_i = np.arange(TPC)[:, None, None, None]
        b_i = np.arange(P)[None, :, None, None]
        t_i = np.arange(T)[None, None, :, None]
        c_i = nn_t // P
        p_i = nn_t % P
        np.add.at(
            A,
            (
                np.broadcast_to(j_i, nn_t.shape),
                np.broadcast_to(t_i, nn_t.shape),
                c_i,
                p_i,
                np.broadcast_to(b_i, nn_t.shape),
            ),
            1,
        )
        cnts = np.ascontiguousarray(
            A.transpose(3, 0, 1, 2, 4).reshape(P, -1)
        ).astype(CNT_NP)
        s1_all = np.ascontiguousarray(
            trans_weights_s1[ct].transpose(1, 0, 2).reshape(U, TPC * DIM_A)
        ).astype(NPBF16)
        w_all = np.ascontiguousarray(
            trans_weights[ct].transpose(1, 0, 2).reshape(U, TPC * EMB)
        ).astype(NPBF16)
        s2_blk = np.ascontiguousarray(trans_weights_s2[ct][:, :, 0].T).astype(NPBF16)
        in_maps.append(
            {
                "ttabs": ttabs,
                "cnts": cnts,
                "ntab": ntab,
                "neidx": ne_idx,
                "s1w": s1_all,
                "s2w": s2_blk,
                "ww": w_all,
            }
        )
    return in_maps


def _phase2_prep(region_index, region_segment_ids, slot_of_sample, lne_rows):
    """Per-core dense count matrices cnt2[p, (c, h, s)] over lne rows x 256
    local segments."""
    seg_ids = np.asarray(region_segment_ids).astype(np.int64)
    new_idx = slot_of_sample[np.asarray(region_index).astype(np.int64)]
    NCH = lne_rows // P

    cnt = np.bincount(seg_ids, minlength=NSEG).astype(np.float32)
    recip_all = np.where(cnt > 0, 1.0 / np.maximum(cnt, 1.0), 0.0).astype(np.float32)

    cnt2_l, recip_l = [], []
    for k in range(N_CORES):
        lo = np.searchsorted(seg_ids, k * SEG_PER_CORE)
        hi = np.searchsorted(seg_ids, (k + 1) * SEG_PER_CORE)
        rows = new_idx[lo:hi]
        segs = seg_ids[lo:hi] - k * SEG_PER_CORE  # [0, 256)
        A = np.zeros((NCH, P, 2, P), np.uint16)
        np.add.at(A, (rows // P, rows % P, segs // P, segs % P), 1)
        cnt2_l.append(
            np.ascontiguousarray(
                A.transpose(1, 0, 2, 3).reshape(P, -1)
            ).astype(CNT_NP)
        )
        rc = recip_all[k * SEG_PER_CORE : (k + 1) * SEG_PER_CORE]
        recip_l.append(np.ascontiguousarray(rc.reshape(2, P).T))
    return cnt2_l, recip_l


def _phase2_inmaps(inputs, lne_full, cnt2_l, recip_l):
    fc_w = np.asarray(inputs["fc_w"], np.float32)
    fc_b = np.asarray(inputs["fc_b"], np.float32)
    fcwt = np.ascontiguousarray(fc_w.T).astype(NPBF16)
    fcb = np.broadcast_to(fc_b[None, :], (P, EMBED_SIZE)).copy()
    in_maps = []
    for k in range(N_CORES):
        in_maps.append(
            {
                "lne": lne_full,
                "cnt2": cnt2_l[k],
                "recip": recip_l[k],
                "fcwt": fcwt,
                "fcb": fcb,
            }
        )
    return in_maps


def _run_spmd_retry(nc, in_maps, retries=3, delay=45.0):
    import time as _time

    last = None
    for attempt in range(retries):
        try:
            return run_bass_kernel_spmd(nc, in_maps, list(range(N_CORES)))
        except Exception as e:
            last = e
            if attempt + 1 < retries:
                _time.sleep(delay)
    raise last


_P1_CACHE = {}
_P2_CACHE = {}


def kernel(**inputs) -> np.ndarray:
    train_inputs = np.asarray(inputs["train_inputs"])
    train_types = np.asarray(inputs["train_types"])
    node_neigh = np.asarray(inputs["node_neigh"])
    num_sms = int(inputs["num_sms"])
    max_region = int(inputs["max_region"])

    sample_mat, tile_type, TPC, slot_of_sample = _phase1_prep(
        train_inputs, train_types, node_neigh
    )
    TT = sample_mat.shape[0]

    if TPC not in _P1_CACHE:
        _P1_CACHE[TPC] = build_phase1(TPC)
    nc1 = _P1_CACHE[TPC]
    in_maps1 = _phase1_inmaps(inputs, sample_mat, tile_type, TPC)
    res1 = _run_spmd_retry(nc1, in_maps1).results

    lne_rows = TT * P
    lne_full = np.empty((lne_rows, EMB), NPBF16)
    for k in range(N_CORES):
        shard = res1[k]["lne"].reshape(P, TPC, EMB).transpose(1, 0, 2)
        lne_full[k * TPC * P : (k + 1) * TPC * P] = shard.reshape(TPC * P, EMB)

    cnt2_l, recip_l = _phase2_prep(
        inputs["region_index"], inputs["region_segment_ids"], slot_of_sample,
        lne_rows,
    )
    if TT not in _P2_CACHE:
        _P2_CACHE[TT] = build_phase2(TT)
    nc2 = _P2_CACHE[TT]
    in_maps2 = _phase2_inmaps(inputs, lne_full, cnt2_l, recip_l)
    res2 = _run_spmd_retry(nc2, in_maps2).results

    out = np.concatenate([res2[k]["out"] for k in range(N_CORES)], axis=0)
    return out.reshape(num_sms, max_region, EMBED_SIZE)


# revision 9
# speedup vs baseline: 1.1319x; 1.1319x over previous
"""GATNE model on 8 Trainium2 NeuronCores — count-matrix matmul version.

Both phases replace SWDGE gathers with dense count-matrix matmuls:
  Phase 1: nte[b,t,:] = sum_c cnt[j,t,c][node,b]^T @ ttab_t[chunk c]  (PE),
  where cnt are host-built dense bf16 count matrices (counts of each node in
  each sample's type-t neighbor list). Only the per-sample node_embeddings
  lookup stays a SWDGE gather (512B bf16 rows). Attention + normalize as in
  the bf16 kernel; output bf16.
  Phase 2: seg_sum[s,:] = sum_c cnt2[c][row,s]^T @ lne[chunk c] (PE), with
  lne streamed densely chunk-group by chunk-group; then mean + normalize +
  FC + l2norm.
"""
import contextlib
import sys

sys.path.insert(0, "/opt/trn_rl_repo")

import ml_dtypes
import numpy as np

import concourse.bacc as bacc
import concourse.bass as bass
import concourse.mybir as mybir
import concourse.tile as tile
from concourse.bass_utils import run_bass_kernel_spmd
from concourse.masks import make_identity

F32 = mybir.dt.float32
BF16 = mybir.dt.bfloat16
I16 = mybir.dt.int16
AF = mybir.ActivationFunctionType
ALU = mybir.AluOpType
NPBF16 = ml_dtypes.bfloat16
# Count matrices ride as fp8e4m3: the counts are small integers (<= 10),
# exactly representable, and the PE accepts fp8 lhsT with bf16 rhs.
CNT_DT = mybir.dt.float8e4
CNT_NP = ml_dtypes.float8_e4m3

N_CORES = 8
NUM_NODES = 2500
T = 4
EMB = 256
U = 64
DIM_A = 32
EMBED_SIZE = 512
NEIGH = 10
B = NUM_NODES * T
NUM_SMS = 64
MAX_REGION = 32
NSEG = NUM_SMS * MAX_REGION  # 2048
SEG_PER_CORE = NSEG // N_CORES  # 256
E_TOTAL = 131072
P = 128
NCH1 = 20  # node chunks (2560 >= 2500)


def _wrap16(flat):
    n = flat.shape[0]
    assert n % 16 == 0
    a = flat.reshape(n // 16, 16).T.astype(np.int16)
    return np.tile(a, (8, 1))


def _split_drain_waits(nc, max_waits=1):
    for bb in nc.main_func.blocks:
        out = []
        for ins in bb.instructions:
            si = ins.sync_info
            if (
                type(ins).__name__ == "InstDrain"
                and si is not None
                and si.on_wait is not None
                and len(si.on_wait) > max_waits
            ):
                waits = list(si.on_wait)
                extra, keep = waits[:-max_waits], waits[-max_waits:]
                for i in range(0, len(extra), max_waits):
                    d = mybir.InstDrain(
                        name=nc.get_next_instruction_name(),
                        ins=[],
                        outs=[],
                        bass_is_fusable=False,
                    )
                    d.engine = ins.engine
                    d.sync_info = mybir.SyncInfo(
                        on_wait=extra[i : i + max_waits], on_update=[]
                    )
                    out.append(d)
                ins.sync_info = mybir.SyncInfo(
                    on_wait=keep, on_update=list(si.on_update or [])
                )
            out.append(ins)
        bb.instructions = out


# ---------------------------------------------------------------------------
# Phase-1 program builder
# ---------------------------------------------------------------------------
def build_phase1(TPC, reps=1):
    nc = bacc.Bacc("TRN2", debug=False)
    ttabs = nc.dram_tensor("ttabs", [P, NCH1 * T * U], BF16, kind="ExternalInput")
    cnts = nc.dram_tensor(
        "cnts", [P, TPC * T * NCH1 * P], CNT_DT, kind="ExternalInput"
    )
    ntab = nc.dram_tensor("ntab", [NUM_NODES, EMB], BF16, kind="ExternalInput")
    neidx = nc.dram_tensor("neidx", [P, TPC * 8], I16, kind="ExternalInput")
    s1w = nc.dram_tensor("s1w", [U, TPC * DIM_A], BF16, kind="ExternalInput")
    s2w = nc.dram_tensor("s2w", [DIM_A, TPC], BF16, kind="ExternalInput")
    ww = nc.dram_tensor("ww", [U, TPC * EMB], BF16, kind="ExternalInput")
    lne_out = nc.dram_tensor("lne", [P, TPC * EMB], BF16, kind="ExternalOutput")

    with tile.TileContext(nc) as tc:
        with (
            tc.tile_pool(name="const", bufs=1) as cpool,
            tc.tile_pool(name="cnt", bufs=3) as cntpool,
            tc.tile_pool(name="work", bufs=2) as wpool,
            tc.tile_pool(name="ps_nte", bufs=2, space="PSUM") as ps_nte,
            tc.tile_pool(name="ps_tp", bufs=2, space="PSUM") as ps_tp,
            tc.tile_pool(name="ps_h", bufs=1, space="PSUM") as ps_h,
            tc.tile_pool(name="ps_lg", bufs=1, space="PSUM") as ps_lg,
            tc.tile_pool(name="ps_dl", bufs=1, space="PSUM") as ps_dl,
        ):
            identb = cpool.tile([P, P], BF16)
            make_identity(nc, identb[:])
            ident = cpool.tile([P, P], F32)
            make_identity(nc, ident[:])
            ttabs_sb = cpool.tile([P, NCH1, T, U], BF16)
            nc.sync.dma_start(
                out=ttabs_sb[:].rearrange("p c t u -> p (c t u)"), in_=ttabs[:]
            )
            s1_sb = cpool.tile([U, TPC * DIM_A], BF16)
            nc.sync.dma_start(out=s1_sb[:], in_=s1w[:])
            s2_sb = cpool.tile([DIM_A, TPC], BF16)
            nc.sync.dma_start(out=s2_sb[:], in_=s2w[:])
            w_sb = cpool.tile([U, TPC * EMB], BF16)
            nc.sync.dma_start(out=w_sb[:], in_=ww[:])
            ne_idx_sb = cpool.tile([P, TPC * 8], I16)
            nc.sync.dma_start(out=ne_idx_sb[:], in_=neidx[:])
            lne_sb = cpool.tile([P, TPC * EMB], BF16)

            with tc.For_i(0, reps, 1) if reps > 1 else contextlib.nullcontext():
                ne_g = cpool.tile([P, TPC, EMB], BF16, tag="ne_g")
                nc.gpsimd.dma_gather(
                    ne_g[:], ntab[:], ne_idx_sb[:], TPC * P, TPC * P, EMB,
                    single_packet=False,
                )

                def stage_a(j):
                    cnt = cntpool.tile([P, T * NCH1, P], CNT_DT, tag="cnt")
                    nc.sync.dma_start(
                        out=cnt[:].rearrange("p a b -> p (a b)"),
                        in_=cnts[:, j * (T * NCH1 * P) : (j + 1) * (T * NCH1 * P)],
                    )
                    nte_ps = ps_nte.tile([P, T * U], F32, tag="nte_ps")
                    for t in range(T):
                        for c in range(NCH1):
                            nc.tensor.matmul(
                                out=nte_ps[:, t * U : (t + 1) * U],
                                lhsT=cnt[:, t * NCH1 + c, :],
                                rhs=ttabs_sb[:, c, t, :],
                                start=(c == 0),
                                stop=(c == NCH1 - 1),
                            )
                    nteb = wpool.tile([P, T * U], BF16, tag="nteb")
                    nc.scalar.activation(nteb[:], nte_ps[:], AF.Copy)
                    pt = ps_tp.tile([U, T * P], BF16, tag="pt")
                    for t in range(T):
                        nc.tensor.transpose(
                            pt[:, t * P : (t + 1) * P],
                            nteb[:, t * U : (t + 1) * U],
                            identb[:],
                        )
                    ntet = wpool.tile([U, T * P], BF16, tag="ntet")
                    nc.vector.tensor_copy(out=ntet[:], in_=pt[:])
                    return ntet

                def stage_b(j, ntet):
                    h_ps = ps_h.tile([DIM_A, T * P], F32, tag="h_ps")
                    nc.tensor.matmul(
                        out=h_ps[:],
                        lhsT=s1_sb[:, j * DIM_A : (j + 1) * DIM_A],
                        rhs=ntet[:],
                        start=True,
                        stop=True,
                    )
                    h = wpool.tile([DIM_A, T * P], BF16, tag="h")
                    nc.scalar.activation(h[:], h_ps[:], AF.Tanh)
                    lg_ps = ps_lg.tile([P, T], F32, tag="lg_ps")
                    for t in range(T):
                        nc.tensor.matmul(
                            out=lg_ps[:, t : t + 1],
                            lhsT=h[:, t * P : (t + 1) * P],
                            rhs=s2_sb[:, j : j + 1],
                            start=True,
                            stop=True,
                        )
                    d_ps = ps_dl.tile([P, T, EMB], F32, tag="d_ps")
                    for t in range(T):
                        nc.tensor.matmul(
                            out=d_ps[:, t, :],
                            lhsT=ntet[:, t * P : (t + 1) * P],
                            rhs=w_sb[:, j * EMB : (j + 1) * EMB],
                            start=True,
                            stop=True,
                        )
                    ex = wpool.tile([P, T], F32, tag="ex")
                    nc.scalar.activation(ex[:], lg_ps[:], AF.Exp)
                    ssum = wpool.tile([P, 1], F32, tag="ssum")
                    nc.vector.tensor_reduce(
                        out=ssum[:], in_=ex[:], axis=mybir.AxisListType.X, op=ALU.add
                    )
                    rs = wpool.tile([P, 1], F32, tag="rs")
                    nc.vector.reciprocal(rs[:], ssum[:])
                    att = wpool.tile([P, T], F32, tag="att")
                    nc.vector.tensor_scalar_mul(att[:], ex[:], rs[:, 0:1])
                    na = wpool.tile([P, EMB], F32, tag="na")
                    nc.vector.scalar_tensor_tensor(
                        out=na[:], in0=d_ps[:, 0, :], scalar=att[:, 0:1],
                        in1=ne_g[:, j, :], op0=ALU.mult, op1=ALU.add,
                    )
                    nb = wpool.tile([P, EMB], F32, tag="nb")
                    nc.vector.scalar_tensor_tensor(
                        out=nb[:], in0=d_ps[:, 1, :], scalar=att[:, 1:2],
                        in1=na[:], op0=ALU.mult, op1=ALU.add,
                    )
                    nc2_ = wpool.tile([P, EMB], F32, tag="nc2")
                    nc.vector.scalar_tensor_tensor(
                        out=nc2_[:], in0=d_ps[:, 2, :], scalar=att[:, 2:3],
                        in1=nb[:], op0=ALU.mult, op1=ALU.add,
                    )
                    nen = wpool.tile([P, EMB], F32, tag="nen")
                    nc.vector.scalar_tensor_tensor(
                        out=nen[:], in0=d_ps[:, 3, :], scalar=att[:, 3:4],
                        in1=nc2_[:], op0=ALU.mult, op1=ALU.add,
                    )
                    sq = wpool.tile([P, EMB], F32, tag="sq")
                    ss = wpool.tile([P, 1], F32, tag="ss")
                    nc.scalar.activation(sq[:], nen[:], AF.Square, accum_out=ss[:])
                    nrm = wpool.tile([P, 1], F32, tag="nrm")
                    nc.scalar.activation(nrm[:], ss[:], AF.Sqrt)
                    nc.vector.tensor_scalar_max(nrm[:], nrm[:], 1e-12)
                    inv = wpool.tile([P, 1], F32, tag="inv")
                    nc.vector.reciprocal(inv[:], nrm[:])
                    nc.vector.tensor_scalar_mul(
                        lne_sb[:, j * EMB : (j + 1) * EMB], nen[:], inv[:, 0:1]
                    )

                prev = None
                for j in range(TPC):
                    ntet_j = stage_a(j)
                    if prev is not None:
                        stage_b(prev[0], prev[1])
                    prev = (j, ntet_j)
                stage_b(prev[0], prev[1])
            nc.sync.dma_start(out=lne_out[:], in_=lne_sb[:])

    nc.compile()
    _split_drain_waits(nc)
    return nc


# ---------------------------------------------------------------------------
# Phase-2 program builder
# ---------------------------------------------------------------------------
def build_phase2(NCH, reps=1):
    """Segment-sum via dense count matmuls over a partition-major lne table
    (lne[p, c*EMB:(c+1)*EMB] = row c*128+p)."""
    GRP = 8
    nc = bacc.Bacc("TRN2", debug=False)
    lne = nc.dram_tensor("lne", [P, NCH * EMB], BF16, kind="ExternalInput")
    cnt2 = nc.dram_tensor("cnt2", [P, NCH * 2 * P], CNT_DT, kind="ExternalInput")
    recip = nc.dram_tensor("recip", [P, 2], F32, kind="ExternalInput")
    fcwt = nc.dram_tensor("fcwt", [EMB, EMBED_SIZE], BF16, kind="ExternalInput")
    fcb = nc.dram_tensor("fcb", [P, EMBED_SIZE], F32, kind="ExternalInput")
    out = nc.dram_tensor("out", [SEG_PER_CORE, EMBED_SIZE], F32, kind="ExternalOutput")

    with tile.TileContext(nc) as tc:
        with (
            tc.tile_pool(name="const", bufs=1) as cpool,
            tc.tile_pool(name="lg", bufs=4) as lgpool,
            tc.tile_pool(name="cg", bufs=4) as cgpool,
            tc.tile_pool(name="work", bufs=2) as wpool,
            tc.tile_pool(name="ps_acc", bufs=2, space="PSUM") as ps_acc,
            tc.tile_pool(name="ps_tp", bufs=2, space="PSUM") as ps_tp,
            tc.tile_pool(name="ps_fc", bufs=2, space="PSUM") as ps_fc,
        ):
            identb = cpool.tile([P, P], BF16)
            make_identity(nc, identb[:])
            recip_sb = cpool.tile([P, 2, 1], F32)
            nc.sync.dma_start(out=recip_sb[:, :, 0], in_=recip[:])
            fcwt0 = cpool.tile([P, EMBED_SIZE], BF16, tag="fcwt0")
            fcwt1 = cpool.tile([P, EMBED_SIZE], BF16, tag="fcwt1")
            fcwt_sb = [fcwt0, fcwt1]
            for i in range(2):
                nc.sync.dma_start(out=fcwt_sb[i][:], in_=fcwt[i * P : (i + 1) * P, :])
            fcb_sb = cpool.tile([P, EMBED_SIZE], F32)
            nc.sync.dma_start(out=fcb_sb[:], in_=fcb[:])
            eps8 = cpool.tile([P, 1], F32)
            nc.vector.memset(eps8[:], 1e-8)

            with tc.For_i(0, reps, 1) if reps > 1 else contextlib.nullcontext():
                acc = ps_acc.tile([P, 2, EMB], F32, tag="acc")
                nc.vector.memset(acc[:], 0.0)
                done = 0
                while done < NCH:
                    ng = min(GRP, NCH - done)
                    lg = lgpool.tile([P, GRP, EMB], BF16, tag="lg")
                    nc.sync.dma_start(
                        out=lg[:, :ng, :].rearrange("p a e -> p (a e)"),
                        in_=lne[:, done * EMB : (done + ng) * EMB],
                    )
                    cg = cgpool.tile([P, GRP, 2, P], CNT_DT, tag="cg")
                    nc.sync.dma_start(
                        out=cg[:, :ng, :, :].rearrange("p a b s -> p (a b s)"),
                        in_=cnt2[:, done * 2 * P : (done + ng) * 2 * P],
                    )
                    for cc in range(ng):
                        c = done + cc
                        for h in range(2):
                            nc.tensor.matmul(
                                out=acc[:, h, :],
                                lhsT=cg[:, cc, h, :],
                                rhs=lg[:, cc, :],
                                start=False,
                                stop=False,
                                skip_group_check=True,
                            )
                    done += ng
                # ---- batched mean + normalize -> sm_all [128, 2, 256] bf16
                mean_all = wpool.tile([P, 2, EMB], F32, tag="mean_all")
                nc.vector.tensor_tensor(
                    out=mean_all[:],
                    in0=acc[:],
                    in1=recip_sb[:].to_broadcast([P, 2, EMB]),
                    op=ALU.mult,
                )
                sq = wpool.tile([P, 2, EMB], F32, tag="p2sq")
                nc.scalar.activation(sq[:], mean_all[:], AF.Square)
                ssn = wpool.tile([P, 2], F32, tag="p2ss")
                nc.vector.tensor_reduce(
                    out=ssn[:], in_=sq[:], axis=mybir.AxisListType.X, op=ALU.add
                )
                nrm = wpool.tile([P, 2], F32, tag="p2nrm")
                nc.scalar.activation(nrm[:], ssn[:], AF.Sqrt)
                nc.vector.tensor_scalar_max(nrm[:], nrm[:], 1e-12)
                inv = wpool.tile([P, 2, 1], F32, tag="p2inv")
                nc.vector.reciprocal(inv[:, :, 0], nrm[:])
                sm_all = wpool.tile([P, 2, EMB], BF16, tag="sm_all")
                nc.vector.tensor_tensor(
                    out=sm_all[:],
                    in0=mean_all[:],
                    in1=inv[:].to_broadcast([P, 2, EMB]),
                    op=ALU.mult,
                )
                # ---- transpose -> smnT [emb-half][128, 256(seg)] bf16
                smnT = []
                for eh in range(2):
                    tp = ps_tp.tile([P, 2 * P], BF16, tag="tp")
                    for half in range(2):
                        nc.tensor.transpose(
                            tp[:, half * P : (half + 1) * P],
                            sm_all[:, half, eh * P : (eh + 1) * P],
                            identb[:],
                        )
                    st = wpool.tile([P, 2 * P], BF16, tag=f"smnT{eh}")
                    nc.vector.tensor_copy(out=st[:], in_=tp[:])
                    smnT.append(st)
                # ---- FC + bias + l2norm
                for m in range(2):
                    fc_ps = ps_fc.tile([P, EMBED_SIZE], F32, tag="fc_ps")
                    for kh in range(2):
                        nc.tensor.matmul(
                            out=fc_ps[:],
                            lhsT=smnT[kh][:, m * P : (m + 1) * P],
                            rhs=fcwt_sb[kh][:],
                            start=(kh == 0),
                            stop=(kh == 1),
                        )
                    xx = wpool.tile([P, EMBED_SIZE], F32, tag="xx")
                    nc.vector.tensor_add(out=xx[:], in0=fc_ps[:], in1=fcb_sb[:])
                    sq3 = wpool.tile([P, EMBED_SIZE], F32, tag="p3sq")
                    ss3 = wpool.tile([P, 1], F32, tag="p3ss")
                    nc.scalar.activation(sq3[:], xx[:], AF.Square, accum_out=ss3[:])
                    nrm3 = wpool.tile([P, 1], F32, tag="p3nrm")
                    nc.scalar.activation(nrm3[:], ss3[:], AF.Sqrt, bias=eps8[:, 0:1])
                    nc.vector.tensor_scalar_add(nrm3[:], nrm3[:], 1e-8)
                    inv3 = wpool.tile([P, 1], F32, tag="p3inv")
                    nc.vector.reciprocal(inv3[:], nrm3[:])
                    res = wpool.tile([P, EMBED_SIZE], F32, tag="res")
                    nc.vector.tensor_scalar_mul(res[:], xx[:], inv3[:, 0:1])
                    nc.sync.dma_start(
                        out=out[m * P : (m + 1) * P, :], in_=res[:]
                    )

    nc.compile()
    _split_drain_waits(nc)
    return nc


# ---------------------------------------------------------------------------
# Host-side orchestration
# ---------------------------------------------------------------------------
def _phase1_prep(train_inputs, train_types, node_neigh):
    order = np.argsort(train_types, kind="stable")
    ts = train_types[order]
    tiles_s, tiles_t = [], []
    for t in range(T):
        idx_t = order[ts == t]
        if len(idx_t) == 0:
            continue
        n_tiles = -(-len(idx_t) // P)
        padded = np.concatenate(
            [idx_t, np.repeat(idx_t[-1:], n_tiles * P - len(idx_t))]
        )
        for jj in range(n_tiles):
            tiles_s.append(padded[jj * P : (jj + 1) * P])
            tiles_t.append(t)
    while len(tiles_s) % N_CORES:
        tiles_s.append(tiles_s[-1])
        tiles_t.append(tiles_t[-1])
    sample_mat = np.stack(tiles_s)
    tile_type = np.asarray(tiles_t)
    TT = sample_mat.shape[0]
    TPC = TT // N_CORES

    flat = sample_mat.reshape(-1)
    slot_of_sample = np.zeros(B, np.int64)
    slot_of_sample[flat[::-1]] = np.arange(TT * P)[::-1]
    return sample_mat, tile_type, TPC, slot_of_sample


def _phase1_inmaps(inputs, sample_mat, tile_type, TPC):
    node_embeddings = np.asarray(inputs["node_embeddings"], np.float32)
    node_type_embeddings = np.asarray(inputs["node_type_embeddings"], np.float32)
    trans_weights = np.asarray(inputs["trans_weights"], np.float32)
    trans_weights_s1 = np.asarray(inputs["trans_weights_s1"], np.float32)
    trans_weights_s2 = np.asarray(inputs["trans_weights_s2"], np.float32)
    train_inputs = np.asarray(inputs["train_inputs"])
    node_neigh = np.asarray(inputs["node_neigh"])

    # ttabs[p, c, t, u] = node_type_embeddings[c*128+p, t, u] (0-pad)
    nte_pad = np.zeros((NCH1 * P, T, U), np.float32)
    nte_pad[:NUM_NODES] = node_type_embeddings
    ttabs = np.ascontiguousarray(
        nte_pad.reshape(NCH1, P, T, U).transpose(1, 0, 2, 3).reshape(P, -1)
    ).astype(NPBF16)
    ntab = node_embeddings.astype(NPBF16)

    in_maps = []
    for k in range(N_CORES):
        smp = sample_mat[k * TPC : (k + 1) * TPC]  # [TPC, 128]
        ct = tile_type[k * TPC : (k + 1) * TPC]
        ne_flat = train_inputs[smp].reshape(-1)
        ne_idx = _wrap16(ne_flat)
        nn_t = node_neigh[smp]  # [TPC, 128, 4, 10]
        # counts A[j, t, c, p_node, b]
        A = np.zeros((TPC, T, NCH1, P, P), np.uint8)
        j_i = np.arange(TPC)[:, None, None, None]
        b_i = np.arange(P)[None, :, None, None]
        t_i = np.arange(T)[None, None, :, None]
        c_i = nn_t // P
        p_i = nn_t % P
        np.add.at(
            A,
            (
                np.broadcast_to(j_i, nn_t.shape),
                np.broadcast_to(t_i, nn_t.shape),
                c_i,
                p_i,
                np.broadcast_to(b_i, nn_t.shape),
            ),
            1,
        )
        cnts = np.ascontiguousarray(
            A.transpose(3, 0, 1, 2, 4).reshape(P, -1)
        ).astype(CNT_NP)
        s1_all = np.ascontiguousarray(
            trans_weights_s1[ct].transpose(1, 0, 2).reshape(U, TPC * DIM_A)
        ).astype(NPBF16)
        w_all = np.ascontiguousarray(
            trans_weights[ct].transpose(1, 0, 2).reshape(U, TPC * EMB)
        ).astype(NPBF16)
        s2_blk = np.ascontiguousarray(trans_weights_s2[ct][:, :, 0].T).astype(NPBF16)
        in_maps.append(
            {
                "ttabs": ttabs,
                "cnts": cnts,
                "ntab": ntab,
                "neidx": ne_idx,
                "s1w": s1_all,
                "s2w": s2_blk,
                "ww": w_all,
            }
        )
    return in_maps


def _phase2_prep(region_index, region_segment_ids, slot_of_sample, lne_rows):
    """Per-core dense count matrices cnt2[p, (c, h, s)] over lne rows x 256
    local segments."""
    seg_ids = np.asarray(region_segment_ids).astype(np.int64)
    new_idx = slot_of_sample[np.asarray(region_index).astype(np.int64)]
    NCH = lne_rows // P

    cnt = np.bincount(seg_ids, minlength=NSEG).astype(np.float32)
    recip_all = np.where(cnt > 0, 1.0 / np.maximum(cnt, 1.0), 0.0).astype(np.float32)

    cnt2_l, recip_l = [], []
    for k in range(N_CORES):
        lo = np.searchsorted(seg_ids, k * SEG_PER_CORE)
        hi = np.searchsorted(seg_ids, (k + 1) * SEG_PER_CORE)
        rows = new_idx[lo:hi]
        segs = seg_ids[lo:hi] - k * SEG_PER_CORE  # [0, 256)
        A = np.zeros((NCH, P, 2, P), np.uint16)
        np.add.at(A, (rows // P, rows % P, segs // P, segs % P), 1)
        cnt2_l.append(
            np.ascontiguousarray(
                A.transpose(1, 0, 2, 3).reshape(P, -1)
            ).astype(CNT_NP)
        )
        rc = recip_all[k * SEG_PER_CORE : (k + 1) * SEG_PER_CORE]
        recip_l.append(np.ascontiguousarray(rc.reshape(2, P).T))
    return cnt2_l, recip_l


def _phase2_inmaps(inputs, lne_pm, cnt2_l, recip_l):
    fc_w = np.asarray(inputs["fc_w"], np.float32)
    fc_b = np.asarray(inputs["fc_b"], np.float32)
    fcwt = np.ascontiguousarray(fc_w.T).astype(NPBF16)
    fcb = np.broadcast_to(fc_b[None, :], (P, EMBED_SIZE)).copy()
    in_maps = []
    for k in range(N_CORES):
        in_maps.append(
            {
                "lne": lne_pm,
                "cnt2": cnt2_l[k],
                "recip": recip_l[k],
                "fcwt": fcwt,
                "fcb": fcb,
            }
        )
    return in_maps


def _run_spmd_retry(nc, in_maps, retries=3, delay=45.0):
    import time as _time

    last = None
    for attempt in range(retries):
        try:
            return run_bass_kernel_spmd(nc, in_maps, list(range(N_CORES)))
        except Exception as e:
            last = e
            if attempt + 1 < retries:
                _time.sleep(delay)
    raise last


_P1_CACHE = {}
_P2_CACHE = {}


def kernel(**inputs) -> np.ndarray:
    train_inputs = np.asarray(inputs["train_inputs"])
    train_types = np.asarray(inputs["train_types"])
    node_neigh = np.asarray(inputs["node_neigh"])
    num_sms = int(inputs["num_sms"])
    max_region = int(inputs["max_region"])

    sample_mat, tile_type, TPC, slot_of_sample = _phase1_prep(
        train_inputs, train_types, node_neigh
    )
    TT = sample_mat.shape[0]

    if TPC not in _P1_CACHE:
        _P1_CACHE[TPC] = build_phase1(TPC)
    nc1 = _P1_CACHE[TPC]
    in_maps1 = _phase1_inmaps(inputs, sample_mat, tile_type, TPC)
    res1 = _run_spmd_retry(nc1, in_maps1).results

    lne_rows = TT * P
    lne_pm = np.concatenate(
        [np.asarray(res1[k]["lne"]) for k in range(N_CORES)], axis=1
    )

    cnt2_l, recip_l = _phase2_prep(
        inputs["region_index"], inputs["region_segment_ids"], slot_of_sample,
        lne_rows,
    )
    if TT not in _P2_CACHE:
        _P2_CACHE[TT] = build_phase2(TT)
    nc2 = _P2_CACHE[TT]
    in_maps2 = _phase2_inmaps(inputs, lne_pm, cnt2_l, recip_l)
    res2 = _run_spmd_retry(nc2, in_maps2).results

    out = np.concatenate([res2[k]["out"] for k in range(N_CORES)], axis=0)
    return out.reshape(num_sms, max_region, EMBED_SIZE)
